# revision 33
# baseline (speedup 1.0000x reference)
"""DOSLoss kernel for Trainium2, 8 NeuronCores, pure data parallel.

Loss = mean|out-scaled|
     + 0.05 * mean|scaling - scaling_factor|
     + 0.005 * mean|cumsum(out,1) - cumsum(scaled,1)|
     + 0.15 * mean|features - dos_features(x, out*scaling[:,None])|

v9 strategy per core (16384 rows x 400 cols, fp32), 4 tiles per
iteration ([128, 1600] f32 loads per tensor on the SP queue, 32 iters):
 - Pool: d = out - scaled as f16 (one [128,1600] op)
 - PE: per tile, 4 fp32 chunk-transposes of `out` -> PSUM (ACT
   evacuates with fused Abs -> a_t f16), 4 f16 chunk-transposes of d ->
   PSUM (DVE evacuates -> d_t), 4 accumulating mini-matmuls (a_t x
   [x^k, window] weights -> moments), and 4 triangular-trimmed matmuls
   (d_t x cumsum matrix U; chunk cc only streams live cols [128cc:400))
 - reduces: sum|d| one strided XY-reduce per iter (DVE); |cumsum| per
   tile: 2 tiles on ACT (abs-activation accum from PSUM), 2 on DVE
 - software pipeline ladder of depth 3 so no engine waits on same-block
   slow producers; end: bulk fp32 feature math on [128, nt] panels, one
   [128, 8] partial-sum tensor out per core; host combines 8 cores.
"""

from contextlib import ExitStack

import numpy as np

import concourse.bacc as bacc
import concourse.bass as bass
import concourse.mybir as mybir
import concourse.tile as tile
from concourse.bass_utils import run_bass_kernel_spmd

F32 = mybir.dt.float32
F32R = mybir.dt.float32r
F16 = mybir.dt.float16
ALU = mybir.AluOpType
AF = mybir.ActivationFunctionType
AX = mybir.AxisListType

N_DOS = 400
N_CORES = 8
B_FULL = 131072
ROWS = B_FULL // N_CORES  # 16384 rows per core
DX = 20.0 / (N_DOS - 1)
ZERO_IDX = 199
SCALING_W = 0.05
CUMSUM_W = 0.005
FEATURES_W = 0.15

TPI = 4  # tiles per iteration
NCH = 4  # 128-dos chunks per tile (128*3 + 16)

# Column layout of the per-core partial output [128, 8]:
# 0: sum|out-scaled|, 1: sum|cumsum diff|, 2: sum|features-feats|,
# 3: sum|scaling-scaling_factor|; 4-7 unused.


def _chunk_rows(cc: int) -> int:
    return min(128, N_DOS - 128 * cc)


def _u128_np() -> np.ndarray:
    # u[p, cc*400 + n'] = 1 if (128cc + p) <= n'  (inclusive-cumsum matrix,
    # chunked by 128 contraction rows)
    u = np.zeros((128, NCH * N_DOS), np.float16)
    for cc in range(NCH):
        for p in range(_chunk_rows(cc)):
            n = 128 * cc + p
            u[p, cc * N_DOS + n : (cc + 1) * N_DOS] = 1.0
    return u


def _w128_np() -> np.ndarray:
    x = -10.0 + DX * np.arange(N_DOS, dtype=np.float64)
    w = np.zeros((N_DOS, 6), np.float64)
    for k in range(5):
        w[:, k] = x**k
    w[ZERO_IDX - 20 : ZERO_IDX + 20, 5] = 1.0
    wsb = np.zeros((128, NCH * 6), np.float16)
    for cc in range(NCH):
        kk = _chunk_rows(cc)
        wsb[0:kk, cc * 6 : (cc + 1) * 6] = w[
            128 * cc : 128 * cc + kk, :
        ].astype(np.float16)
    return wsb


def build_nc(rows: int = ROWS) -> bass.Bass:
    nt = rows // 128  # row tiles
    nit = nt // TPI  # iterations
    assert nt % TPI == 0
    assert nit >= 4

    nc = bacc.Bacc()
    d_out = nc.dram_tensor("x_out", [rows, N_DOS], F32, kind="ExternalInput")
    d_scaled = nc.dram_tensor("x_scaled", [rows, N_DOS], F32, kind="ExternalInput")
    d_scaling = nc.dram_tensor("x_scaling", [rows], F32, kind="ExternalInput")
    d_sf = nc.dram_tensor("x_sf", [rows], F32, kind="ExternalInput")
    d_feat = nc.dram_tensor("x_feat", [rows, 5], F32, kind="ExternalInput")
    d_w = nc.dram_tensor("w128_const", [128, NCH * 6], F16, kind="ExternalInput")
    d_u = nc.dram_tensor("u128_const", [128, NCH * N_DOS], F16, kind="ExternalInput")
    d_ident = nc.dram_tensor("ident", [128, 128], F32, kind="ExternalInput")
    d_i16 = nc.dram_tensor("ident16", [128, 128], F16, kind="ExternalInput")
    d_res = nc.dram_tensor("partials", [128, 8], F32, kind="ExternalOutput")

    with tile.TileContext(nc) as tc:
        with ExitStack() as ctx:
            const_pool = ctx.enter_context(tc.tile_pool(name="const", bufs=1))
            pers_pool = ctx.enter_context(tc.tile_pool(name="pers", bufs=1))
            io_pool = ctx.enter_context(tc.tile_pool(name="io", bufs=5))
            wk_pool = ctx.enter_context(tc.tile_pool(name="wk", bufs=4))
            at_pool = ctx.enter_context(tc.tile_pool(name="at", bufs=8))
            dt_pool = ctx.enter_context(tc.tile_pool(name="dt", bufs=8))
            scr_pool = ctx.enter_context(tc.tile_pool(name="scr", bufs=2))
            pso_pool = ctx.enter_context(
                tc.tile_pool(name="psO", bufs=3, space="PSUM")
            )
            psd_pool = ctx.enter_context(
                tc.tile_pool(name="psD", bufs=2, space="PSUM")
            )
            cps_pool = ctx.enter_context(
                tc.tile_pool(name="cps", bufs=2, space="PSUM")
            )
            ps6_pool = ctx.enter_context(
                tc.tile_pool(name="ps6", bufs=1, space="PSUM")
            )
            fin_pool = ctx.enter_context(tc.tile_pool(name="fin", bufs=1))

            w128 = const_pool.tile([128, NCH * 6], F16, tag="w128")
            nc.scalar.dma_start(w128[:], d_w[:])
            u128 = const_pool.tile([128, NCH * N_DOS], F16, tag="u128")
            nc.scalar.dma_start(u128[:], d_u[:])
            ident = const_pool.tile([128, 128], F32, tag="ident")
            nc.scalar.dma_start(ident[:], d_ident[:])
            i16 = const_pool.tile([128, 128], F16, tag="i16")
            nc.scalar.dma_start(i16[:], d_i16[:])

            # Dummy PE ops so the PE sequencer observes the const-load DMA
            # lanes before the loop (steady-state matmuls then carry at most
            # one wait each).
            scr_ps = pso_pool.tile([128, 512], F32, tag="psO", name="scr_ps")
            nc.tensor.transpose(scr_ps[:, 0:128], ident[:], ident[:])
            scr_pd = psd_pool.tile([128, 512], F16, tag="psD", name="scr_pd")
            nc.tensor.transpose(scr_pd[:, 0:128], i16[:], i16[:])
            scr_c = cps_pool.tile([128, N_DOS], F32, tag="cps", name="scr_c")
            nc.tensor.matmul(
                scr_c[0:24, 0:6], lhsT=u128[0:100, 0:24], rhs=u128[0:100, 0:6]
            )
            scr_p6 = ps6_pool.tile([128, TPI * 6], F32, tag="ps6", name="scr_p6")
            nc.tensor.matmul(
                scr_p6[0:24, 0:6], lhsT=w128[0:16, 0:24], rhs=w128[0:16, 0:6]
            )

            dsums = pers_pool.tile([128, nit], F32, tag="dsums")
            csums = pers_pool.tile([128, nt], F32, tag="csums")
            s6 = pers_pool.tile([128, nt * 6], F32, tag="s6")

            src_o = d_out.rearrange("(i q j) n -> q i (j n)", q=128, j=TPI)
            src_s = d_scaled.rearrange("(i q j) n -> q i (j n)", q=128, j=TPI)

            sc_t = fin_pool.tile([128, nt], F32, tag="sc_t")
            sc_c = fin_pool.tile([128, nt], F32, tag="sc_c")
            sf_c = fin_pool.tile([128, nt], F32, tag="sf_c")
            fv = fin_pool.tile([128, nt * 5], F32, tag="fv")
            fs = fin_pool.tile([128, 8], F32, tag="fs")

            def emit_aux_loads():
                nc.sync.dma_start(
                    sc_t[:].rearrange("q (i j) -> q i j", j=TPI),
                    d_scaling.rearrange("(i q j) -> q i j", q=128, j=TPI),
                )
                nc.sync.dma_start(
                    sc_c[:], d_scaling.rearrange("(q t) -> q t", t=nt)
                )
                nc.sync.dma_start(sf_c[:], d_sf.rearrange("(q t) -> q t", t=nt))
                nc.sync.dma_start(
                    fv[:].rearrange("q (i j f) -> q i j f", j=TPI, f=5),
                    d_feat.rearrange("(i q j) f -> q i j f", q=128, j=TPI),
                )

            def emit_feature_math(lo, hi, acc_col):
                """|features - feats| partial sum for tile cols [lo, hi)."""
                w_ = hi - lo
                s6v = s6[:].rearrange("q (t k) -> q k t", k=6)[:, :, lo:hi]

                def ftile(tag):
                    return fin_pool.tile(
                        [128, w_], F32, tag=f"{tag}_{lo}", name=f"{tag}_{lo}"
                    )

                r0 = ftile("r0")
                nc.vector.reciprocal(r0[:], s6v[:, 0])
                cc_ = ftile("cc")
                nc.vector.tensor_tensor(cc_[:], s6v[:, 1], r0[:], op=ALU.mult)
                r2 = ftile("r2")
                nc.vector.tensor_tensor(r2[:], s6v[:, 2], r0[:], op=ALU.mult)
                r3 = ftile("r3")
                nc.vector.tensor_tensor(r3[:], s6v[:, 3], r0[:], op=ALU.mult)
                r4 = ftile("r4")
                nc.vector.tensor_tensor(r4[:], s6v[:, 4], r0[:], op=ALU.mult)
                csq = ftile("csq")
                nc.vector.tensor_tensor(csq[:], cc_[:], cc_[:], op=ALU.mult)
                wid = ftile("wid")
                nc.vector.tensor_tensor(wid[:], r2[:], csq[:], op=ALU.subtract)
                rw = ftile("rw")
                nc.vector.reciprocal(rw[:], wid[:])
                sq = ftile("sq")
                nc.scalar.activation(sq[:], rw[:], AF.Sqrt)  # sqrt(1/w)
                rw15 = ftile("rw15")
                nc.vector.tensor_tensor(rw15[:], rw[:], sq[:], op=ALU.mult)
                rw2 = ftile("rw2")
                nc.vector.tensor_tensor(rw2[:], rw[:], rw[:], op=ALU.mult)

                # skew = (r3 - 3 c r2 + 2 c^3) * w^-1.5
                t3 = ftile("t3")
                nc.vector.scalar_tensor_tensor(
                    t3[:], cc_[:], 3.0, r2[:], op0=ALU.mult, op1=ALU.mult
                )
                t4 = ftile("t4")
                nc.vector.tensor_tensor(t4[:], r3[:], t3[:], op=ALU.subtract)
                c3 = ftile("c3")
                nc.vector.tensor_tensor(c3[:], csq[:], cc_[:], op=ALU.mult)
                skn = ftile("skn")
                nc.vector.scalar_tensor_tensor(
                    skn[:], c3[:], 2.0, t4[:], op0=ALU.mult, op1=ALU.add
                )
                skew = ftile("skew")
                nc.vector.tensor_tensor(skew[:], skn[:], rw15[:], op=ALU.mult)

                # kurt = (r4 - 4 c r3 + 6 c^2 r2 - 3 c^4) * w^-2
                u1 = ftile("u1")
                nc.vector.scalar_tensor_tensor(
                    u1[:], cc_[:], 4.0, r3[:], op0=ALU.mult, op1=ALU.mult
                )
                u2 = ftile("u2")
                nc.vector.tensor_tensor(u2[:], r4[:], u1[:], op=ALU.subtract)
                u3 = ftile("u3")
                nc.vector.scalar_tensor_tensor(
                    u3[:], csq[:], 6.0, r2[:], op0=ALU.mult, op1=ALU.mult
                )
                u4 = ftile("u4")
                nc.vector.tensor_tensor(u4[:], u2[:], u3[:], op=ALU.add)
                u5 = ftile("u5")
                nc.vector.scalar_tensor_tensor(
                    u5[:], csq[:], 3.0, csq[:], op0=ALU.mult, op1=ALU.mult
                )
                kn = ftile("kn")
                nc.vector.tensor_tensor(kn[:], u4[:], u5[:], op=ALU.subtract)
                kurt = ftile("kurt")
                nc.vector.tensor_tensor(kurt[:], kn[:], rw2[:], op=ALU.mult)

                ef = ftile("ef")
                nc.vector.scalar_tensor_tensor(
                    ef[:], s6v[:, 5], DX, sc_t[:, lo:hi],
                    op0=ALU.mult, op1=ALU.mult,
                )

                fdiff = fin_pool.tile(
                    [128, w_ * 5], F32, tag=f"fdiff_{lo}", name=f"fdiff_{lo}"
                )
                fdv = fdiff[:].rearrange("q (t f) -> q f t", f=5)
                fvv = fv[:].rearrange("q (t f) -> q f t", f=5)[:, :, lo:hi]
                feats = [cc_, wid, skew, kurt, ef]
                for kf in range(5):
                    nc.vector.tensor_tensor(
                        fdv[:, kf], fvv[:, kf], feats[kf][:], op=ALU.subtract
                    )
                scr_f = fin_pool.tile(
                    [128, w_ * 5], F32, tag=f"scrF_{lo}", name=f"scrF_{lo}"
                )
                nc.scalar.activation(
                    scr_f[:], fdiff[:], AF.Abs,
                    accum_out=fs[:, acc_col : acc_col + 1],
                )

            o4s: dict = {}
            s4s: dict = {}
            d4s: dict = {}
            oa4s: dict = {}
            ats: dict = {}
            dts: dict = {}

            for i in range(nit + 5):
                # ---- Pool (data i-2): d = out - scaled as f16 ----
                j = i - 2
                if 0 <= j < nit:
                    hw = TPI * N_DOS // 2
                    d4 = wk_pool.tile([128, TPI * N_DOS], F16, tag="d4")
                    oa4 = wk_pool.tile([128, TPI * N_DOS], F16, tag="oa4")
                    for hh in range(2):
                        sl = slice(hh * hw, (hh + 1) * hw)
                        nc.gpsimd.tensor_tensor(
                            d4[:, sl], o4s[j][:, sl], s4s[j][:, sl],
                            op=ALU.subtract,
                        )
                        nc.scalar.activation(oa4[:, sl], o4s[j][:, sl], AF.Abs)
                    d4s[j] = d4
                    oa4s[j] = oa4

                # ---- PE transposes (data i-3) + ACT/DVE evacuations ----
                k = i - 3
                if 0 <= k < nit:
                    oa4 = oa4s[k]
                    a_ts = []
                    for t in range(TPI):
                        p_o = pso_pool.tile([128, 512], F16, tag="psO")
                        for cc in range(NCH):
                            kk = _chunk_rows(cc)
                            nc.tensor.transpose(
                                p_o[0:kk, cc * 128 : cc * 128 + 128],
                                oa4[
                                    :,
                                    t * N_DOS + 128 * cc : t * N_DOS + 128 * cc + kk,
                                ],
                                i16[:],
                            )
                        a_t = at_pool.tile([128, 512], F16, tag="a_t")
                        if t < 4:
                            nc.scalar.copy(a_t[:], p_o[:])
                        else:
                            nc.vector.tensor_copy(a_t[:], p_o[:])
                        a_ts.append(a_t)
                    ats[k] = a_ts
                    d4 = d4s[k]
                    d_ts = []
                    for t in range(TPI):
                        p_d = psd_pool.tile([128, 512], F16, tag="psD")
                        for cc in range(NCH):
                            kk = _chunk_rows(cc)
                            nc.tensor.transpose(
                                p_d[0:kk, cc * 128 : cc * 128 + 128],
                                d4[:, t * N_DOS + 128 * cc : t * N_DOS + 128 * cc + kk],
                                i16[:],
                            )
                        d_t = dt_pool.tile([128, 512], F16, tag="d_t")
                        nc.vector.tensor_copy(d_t[:], p_d[:])
                        d_ts.append(d_t)
                    dts[k] = d_ts

                # ---- PE matmuls + reduces (data i-4) ----
                m = i - 4
                if 0 <= m < nit:
                    a_ts = ats.pop(m)
                    ps6 = ps6_pool.tile([128, TPI * 6], F32, tag="ps6")
                    for t in range(TPI):
                        for cc in range(NCH):
                            kk = _chunk_rows(cc)
                            nc.tensor.matmul(
                                ps6[:, 6 * t : 6 * t + 6],
                                lhsT=a_ts[t][0:kk, cc * 128 : cc * 128 + 128],
                                rhs=w128[0:kk, 6 * cc : 6 * cc + 6],
                                start=(cc == 0),
                                stop=(cc == NCH - 1),
                            )
                    d_ts = dts.pop(m)
                    cpss = []
                    for t in range(TPI):
                        c_ps = cps_pool.tile([128, N_DOS], F32, tag="cps")
                        for cc in range(NCH):
                            kk = _chunk_rows(cc)
                            lo = 128 * cc
                            nc.tensor.matmul(
                                c_ps[:, lo:N_DOS],
                                lhsT=d_ts[t][0:kk, cc * 128 : cc * 128 + 128],
                                rhs=u128[0:kk, cc * N_DOS + lo : (cc + 1) * N_DOS],
                                start=(cc == 0),
                                stop=(cc == NCH - 1),
                                skip_group_check=True,
                            )
                        cpss.append(c_ps)
                    nc.scalar.copy(s6[:, 6 * TPI * m : 6 * TPI * (m + 1)], ps6[:])
                    for t in range(TPI):
                        col = TPI * m + t
                        if t < 2:
                            scr_a = scr_pool.tile(
                                [128, N_DOS], F16, tag="scrA", name="scrA"
                            )
                            nc.scalar.activation(
                                scr_a[:],
                                cpss[t][:],
                                AF.Abs,
                                accum_out=csums[:, col : col + 1],
                            )
                        else:
                            nc.vector.tensor_reduce(
                                csums[:, col : col + 1],
                                cpss[t][:],
                                axis=AX.X,
                                op=ALU.add,
                                apply_absolute_value=True,
                            )
                    d4v = d4s[m][:].rearrange("q (j n) -> q j n", j=TPI)
                    nc.vector.tensor_reduce(
                        dsums[:, m : m + 1],
                        d4v[:],
                        axis=AX.XY,
                        op=ALU.add,
                        apply_absolute_value=True,
                    )
                    d4s.pop(m, None)
                    o4s.pop(m, None)
                    s4s.pop(m, None)
                    oa4s.pop(m, None)

                if i == nit - 3:
                    emit_aux_loads()

                # ---- loads (half-tensor granularity for latency) ----
                if i < nit:
                    o4 = io_pool.tile([128, TPI * N_DOS], F32, tag="o4")
                    s4 = io_pool.tile([128, TPI * N_DOS], F32, tag="s4")
                    hw2 = TPI * N_DOS // 2
                    for hh in range(2):
                        sl = slice(hh * hw2, (hh + 1) * hw2)
                        nc.sync.dma_start(o4[:, sl], src_o[:, i, sl])
                        nc.sync.dma_start(s4[:, sl], src_s[:, i, sl])
                    o4s[i] = o4
                    s4s[i] = s4

            # ---- end phase: feature math ----
            emit_feature_math(0, nt, 2)

            dsc = fin_pool.tile([128, nt], F32, tag="dsc")
            nc.vector.tensor_tensor(dsc[:], sc_c[:], sf_c[:], op=ALU.subtract)
            scr_s = fin_pool.tile([128, nt], F32, tag="scrS")
            nc.scalar.activation(
                scr_s[:], dsc[:], AF.Abs, accum_out=fs[:, 3:4]
            )

            nc.vector.tensor_reduce(fs[:, 0:1], dsums[:], axis=AX.X, op=ALU.add)
            nc.vector.tensor_reduce(fs[:, 1:2], csums[:], axis=AX.X, op=ALU.add)
            nc.gpsimd.memset(fs[:, 4:8], 0.0)

            nc.sync.dma_start(d_res[:], fs[:])

    nc.compile()
    return nc


_NC_CACHE: dict = {}


def _get_nc(rows: int) -> bass.Bass:
    if rows not in _NC_CACHE:
        _NC_CACHE[rows] = build_nc(rows)
    return _NC_CACHE[rows]


def make_in_maps(out, scaling, scaled, scaling_factor, features, n_cores=N_CORES):
    rows = out.shape[0] // n_cores
    w128 = _w128_np()
    u128 = _u128_np()
    ident = np.eye(128, dtype=np.float32)
    i16 = np.eye(128, dtype=np.float16)
    in_maps = []
    for i in range(n_cores):
        sl = slice(i * rows, (i + 1) * rows)
        in_maps.append(
            {
                "x_out": np.ascontiguousarray(out[sl]),
                "x_scaled": np.ascontiguousarray(scaled[sl]),
                "x_scaling": np.ascontiguousarray(scaling[sl]),
                "x_sf": np.ascontiguousarray(scaling_factor[sl]),
                "x_feat": np.ascontiguousarray(features[sl]),
                "w128_const": w128,
                "u128_const": u128,
                "ident": ident,
                "ident16": i16,
            }
        )
    return in_maps


def combine_partials(partials_list, b_full: int) -> np.float32:
    tot = np.zeros(5, np.float64)
    for fs in partials_list:
        tot += fs[:, 0:5].astype(np.float64).sum(axis=0)
    dos_loss = tot[0] / (b_full * N_DOS)
    cumsum_loss = tot[1] / (b_full * N_DOS)
    features_loss = (tot[2] + tot[4]) / (b_full * 5)
    scaling_loss = tot[3] / b_full
    return np.float32(
        dos_loss
        + SCALING_W * scaling_loss
        + CUMSUM_W * cumsum_loss
        + FEATURES_W * features_loss
    )


def kernel(out, scaling, scaled, scaling_factor, features):
    out = np.asarray(out, np.float32)
    scaling = np.asarray(scaling, np.float32)
    scaled = np.asarray(scaled, np.float32)
    scaling_factor = np.asarray(scaling_factor, np.float32)
    features = np.asarray(features, np.float32)

    nc = _get_nc(ROWS)
    in_maps = make_in_maps(out, scaling, scaled, scaling_factor, features)
    res = run_bass_kernel_spmd(nc, in_maps, list(range(N_CORES)))
    partials = [res.results[i]["partials"] for i in range(N_CORES)]
    return combine_partials(partials, out.shape[0])


if __name__ == "__main__":
    print("building...")
    nc = build_nc(2048)
    print("instructions built ok")


# revision 34
# speedup vs baseline: 1.0690x; 1.0690x over previous
"""DOSLoss kernel for Trainium2, 8 NeuronCores, pure data parallel.

Loss = mean|out-scaled|
     + 0.05 * mean|scaling - scaling_factor|
     + 0.005 * mean|cumsum(out,1) - cumsum(scaled,1)|
     + 0.15 * mean|features - dos_features(x, out*scaling[:,None])|

v9 strategy per core (16384 rows x 400 cols, fp32), 4 tiles per
iteration ([128, 1600] f32 loads per tensor on the SP queue, 32 iters):
 - Pool: d = out - scaled as f16 (one [128,1600] op)
 - PE: per tile, 4 fp32 chunk-transposes of `out` -> PSUM (ACT
   evacuates with fused Abs -> a_t f16), 4 f16 chunk-transposes of d ->
   PSUM (DVE evacuates -> d_t), 4 accumulating mini-matmuls (a_t x
   [x^k, window] weights -> moments), and 4 triangular-trimmed matmuls
   (d_t x cumsum matrix U; chunk cc only streams live cols [128cc:400))
 - reduces: sum|d| one strided XY-reduce per iter (DVE); |cumsum| per
   tile: 2 tiles on ACT (abs-activation accum from PSUM), 2 on DVE
 - software pipeline ladder of depth 3 so no engine waits on same-block
   slow producers; end: bulk fp32 feature math on [128, nt] panels, one
   [128, 8] partial-sum tensor out per core; host combines 8 cores.
"""

from contextlib import ExitStack

import numpy as np

import concourse.bacc as bacc
import concourse.bass as bass
import concourse.mybir as mybir
import concourse.tile as tile
from concourse.bass_utils import run_bass_kernel_spmd

F32 = mybir.dt.float32
F32R = mybir.dt.float32r
F16 = mybir.dt.float16
ALU = mybir.AluOpType
AF = mybir.ActivationFunctionType
AX = mybir.AxisListType

N_DOS = 400
N_CORES = 8
B_FULL = 131072
ROWS = B_FULL // N_CORES  # 16384 rows per core
DX = 20.0 / (N_DOS - 1)
ZERO_IDX = 199
SCALING_W = 0.05
CUMSUM_W = 0.005
FEATURES_W = 0.15

TPI = 4  # tiles per iteration
NCH = 4  # 128-dos chunks per tile (128*3 + 16)

# Column layout of the per-core partial output [128, 8]:
# 0: sum|out-scaled|, 1: sum|cumsum diff|, 2: sum|features-feats|,
# 3: sum|scaling-scaling_factor|; 4-7 unused.


def _chunk_rows(cc: int) -> int:
    return min(128, N_DOS - 128 * cc)


def _u128_np() -> np.ndarray:
    # u[p, cc*400 + n'] = 1 if (128cc + p) <= n'  (inclusive-cumsum matrix,
    # chunked by 128 contraction rows)
    u = np.zeros((128, NCH * N_DOS), np.float16)
    for cc in range(NCH):
        for p in range(_chunk_rows(cc)):
            n = 128 * cc + p
            u[p, cc * N_DOS + n : (cc + 1) * N_DOS] = 1.0
    return u


def _w128_np() -> np.ndarray:
    x = -10.0 + DX * np.arange(N_DOS, dtype=np.float64)
    w = np.zeros((N_DOS, 6), np.float64)
    for k in range(5):
        w[:, k] = x**k
    w[ZERO_IDX - 20 : ZERO_IDX + 20, 5] = 1.0
    wsb = np.zeros((128, NCH * 6), np.float16)
    for cc in range(NCH):
        kk = _chunk_rows(cc)
        wsb[0:kk, cc * 6 : (cc + 1) * 6] = w[
            128 * cc : 128 * cc + kk, :
        ].astype(np.float16)
    return wsb


def build_nc(rows: int = ROWS) -> bass.Bass:
    nt = rows // 128  # row tiles
    nit = nt // TPI  # iterations
    assert nt % TPI == 0
    assert nit >= 4

    nc = bacc.Bacc()
    d_out = nc.dram_tensor("x_out", [rows, N_DOS], F32, kind="ExternalInput")
    d_scaled = nc.dram_tensor("x_scaled", [rows, N_DOS], F32, kind="ExternalInput")
    d_scaling = nc.dram_tensor("x_scaling", [rows], F32, kind="ExternalInput")
    d_sf = nc.dram_tensor("x_sf", [rows], F32, kind="ExternalInput")
    d_feat = nc.dram_tensor("x_feat", [rows, 5], F32, kind="ExternalInput")
    d_w = nc.dram_tensor("w128_const", [128, NCH * 6], F16, kind="ExternalInput")
    d_u = nc.dram_tensor("u128_const", [128, NCH * N_DOS], F16, kind="ExternalInput")
    d_ident = nc.dram_tensor("ident", [128, 128], F32, kind="ExternalInput")
    d_i16 = nc.dram_tensor("ident16", [128, 128], F16, kind="ExternalInput")
    d_res = nc.dram_tensor("partials", [128, 8], F32, kind="ExternalOutput")

    with tile.TileContext(nc) as tc:
        with ExitStack() as ctx:
            const_pool = ctx.enter_context(tc.tile_pool(name="const", bufs=1))
            pers_pool = ctx.enter_context(tc.tile_pool(name="pers", bufs=1))
            io_pool = ctx.enter_context(tc.tile_pool(name="io", bufs=5))
            wk_pool = ctx.enter_context(tc.tile_pool(name="wk", bufs=4))
            at_pool = ctx.enter_context(tc.tile_pool(name="at", bufs=8))
            dt_pool = ctx.enter_context(tc.tile_pool(name="dt", bufs=8))
            scr_pool = ctx.enter_context(tc.tile_pool(name="scr", bufs=2))
            pso_pool = ctx.enter_context(
                tc.tile_pool(name="psO", bufs=3, space="PSUM")
            )
            psd_pool = ctx.enter_context(
                tc.tile_pool(name="psD", bufs=2, space="PSUM")
            )
            cps_pool = ctx.enter_context(
                tc.tile_pool(name="cps", bufs=2, space="PSUM")
            )
            ps6_pool = ctx.enter_context(
                tc.tile_pool(name="ps6", bufs=1, space="PSUM")
            )
            fin_pool = ctx.enter_context(tc.tile_pool(name="fin", bufs=1))

            w128 = const_pool.tile([128, NCH * 6], F16, tag="w128")
            nc.scalar.dma_start(w128[:], d_w[:])
            u128 = const_pool.tile([128, NCH * N_DOS], F16, tag="u128")
            nc.scalar.dma_start(u128[:], d_u[:])
            ident = const_pool.tile([128, 128], F32, tag="ident")
            nc.scalar.dma_start(ident[:], d_ident[:])
            i16 = const_pool.tile([128, 128], F16, tag="i16")
            nc.scalar.dma_start(i16[:], d_i16[:])

            # Dummy PE ops so the PE sequencer observes the const-load DMA
            # lanes before the loop (steady-state matmuls then carry at most
            # one wait each).
            scr_ps = pso_pool.tile([128, 512], F32, tag="psO", name="scr_ps")
            nc.tensor.transpose(scr_ps[:, 0:128], ident[:], ident[:])
            scr_pd = psd_pool.tile([128, 512], F16, tag="psD", name="scr_pd")
            nc.tensor.transpose(scr_pd[:, 0:128], i16[:], i16[:])
            scr_c = cps_pool.tile([128, N_DOS], F32, tag="cps", name="scr_c")
            nc.tensor.matmul(
                scr_c[0:24, 0:6], lhsT=u128[0:100, 0:24], rhs=u128[0:100, 0:6]
            )
            scr_p6 = ps6_pool.tile([128, TPI * 6], F32, tag="ps6", name="scr_p6")
            nc.tensor.matmul(
                scr_p6[0:24, 0:6], lhsT=w128[0:16, 0:24], rhs=w128[0:16, 0:6]
            )

            dsums = pers_pool.tile([128, nit], F32, tag="dsums")
            csums = pers_pool.tile([128, nt], F32, tag="csums")
            s6 = pers_pool.tile([128, nt * 6], F32, tag="s6")

            src_o = d_out.rearrange("(i q j) n -> q i (j n)", q=128, j=TPI)
            src_s = d_scaled.rearrange("(i q j) n -> q i (j n)", q=128, j=TPI)

            sc_t = fin_pool.tile([128, nt], F32, tag="sc_t")
            sc_c = fin_pool.tile([128, nt], F32, tag="sc_c")
            sf_c = fin_pool.tile([128, nt], F32, tag="sf_c")
            fv = fin_pool.tile([128, nt * 5], F32, tag="fv")
            fs = fin_pool.tile([128, 8], F32, tag="fs")

            def emit_aux_loads():
                nc.sync.dma_start(
                    sc_t[:].rearrange("q (i j) -> q i j", j=TPI),
                    d_scaling.rearrange("(i q j) -> q i j", q=128, j=TPI),
                )
                nc.sync.dma_start(
                    sc_c[:], d_scaling.rearrange("(q t) -> q t", t=nt)
                )
                nc.sync.dma_start(sf_c[:], d_sf.rearrange("(q t) -> q t", t=nt))
                nc.sync.dma_start(
                    fv[:].rearrange("q (i j f) -> q i j f", j=TPI, f=5),
                    d_feat.rearrange("(i q j) f -> q i j f", q=128, j=TPI),
                )

            def emit_feature_math(lo, hi, acc_col):
                """|features - feats| partial sum for tile cols [lo, hi)."""
                w_ = hi - lo
                s6v = s6[:].rearrange("q (t k) -> q k t", k=6)[:, :, lo:hi]

                def ftile(tag):
                    return fin_pool.tile(
                        [128, w_], F32, tag=f"{tag}_{lo}", name=f"{tag}_{lo}"
                    )

                r0 = ftile("r0")
                nc.vector.reciprocal(r0[:], s6v[:, 0])
                cc_ = ftile("cc")
                nc.vector.tensor_tensor(cc_[:], s6v[:, 1], r0[:], op=ALU.mult)
                r2 = ftile("r2")
                nc.vector.tensor_tensor(r2[:], s6v[:, 2], r0[:], op=ALU.mult)
                r3 = ftile("r3")
                nc.vector.tensor_tensor(r3[:], s6v[:, 3], r0[:], op=ALU.mult)
                r4 = ftile("r4")
                nc.vector.tensor_tensor(r4[:], s6v[:, 4], r0[:], op=ALU.mult)
                csq = ftile("csq")
                nc.vector.tensor_tensor(csq[:], cc_[:], cc_[:], op=ALU.mult)
                wid = ftile("wid")
                nc.vector.tensor_tensor(wid[:], r2[:], csq[:], op=ALU.subtract)
                rw = ftile("rw")
                nc.vector.reciprocal(rw[:], wid[:])
                sq = ftile("sq")
                nc.scalar.activation(sq[:], rw[:], AF.Sqrt)  # sqrt(1/w)
                rw15 = ftile("rw15")
                nc.vector.tensor_tensor(rw15[:], rw[:], sq[:], op=ALU.mult)
                rw2 = ftile("rw2")
                nc.vector.tensor_tensor(rw2[:], rw[:], rw[:], op=ALU.mult)

                # skew = (r3 - 3 c r2 + 2 c^3) * w^-1.5
                t3 = ftile("t3")
                nc.vector.scalar_tensor_tensor(
                    t3[:], cc_[:], 3.0, r2[:], op0=ALU.mult, op1=ALU.mult
                )
                t4 = ftile("t4")
                nc.vector.tensor_tensor(t4[:], r3[:], t3[:], op=ALU.subtract)
                c3 = ftile("c3")
                nc.vector.tensor_tensor(c3[:], csq[:], cc_[:], op=ALU.mult)
                skn = ftile("skn")
                nc.vector.scalar_tensor_tensor(
                    skn[:], c3[:], 2.0, t4[:], op0=ALU.mult, op1=ALU.add
                )
                skew = ftile("skew")
                nc.vector.tensor_tensor(skew[:], skn[:], rw15[:], op=ALU.mult)

                # kurt = (r4 - 4 c r3 + 6 c^2 r2 - 3 c^4) * w^-2
                u1 = ftile("u1")
                nc.vector.scalar_tensor_tensor(
                    u1[:], cc_[:], 4.0, r3[:], op0=ALU.mult, op1=ALU.mult
                )
                u2 = ftile("u2")
                nc.vector.tensor_tensor(u2[:], r4[:], u1[:], op=ALU.subtract)
                u3 = ftile("u3")
                nc.vector.scalar_tensor_tensor(
                    u3[:], csq[:], 6.0, r2[:], op0=ALU.mult, op1=ALU.mult
                )
                u4 = ftile("u4")
                nc.vector.tensor_tensor(u4[:], u2[:], u3[:], op=ALU.add)
                u5 = ftile("u5")
                nc.vector.scalar_tensor_tensor(
                    u5[:], csq[:], 3.0, csq[:], op0=ALU.mult, op1=ALU.mult
                )
                kn = ftile("kn")
                nc.vector.tensor_tensor(kn[:], u4[:], u5[:], op=ALU.subtract)
                kurt = ftile("kurt")
                nc.vector.tensor_tensor(kurt[:], kn[:], rw2[:], op=ALU.mult)

                ef = ftile("ef")
                nc.vector.scalar_tensor_tensor(
                    ef[:], s6v[:, 5], DX, sc_t[:, lo:hi],
                    op0=ALU.mult, op1=ALU.mult,
                )

                fdiff = fin_pool.tile(
                    [128, w_ * 5], F32, tag=f"fdiff_{lo}", name=f"fdiff_{lo}"
                )
                fdv = fdiff[:].rearrange("q (t f) -> q f t", f=5)
                fvv = fv[:].rearrange("q (t f) -> q f t", f=5)[:, :, lo:hi]
                feats = [cc_, wid, skew, kurt, ef]
                for kf in range(5):
                    nc.vector.tensor_tensor(
                        fdv[:, kf], fvv[:, kf], feats[kf][:], op=ALU.subtract
                    )
                scr_f = fin_pool.tile(
                    [128, w_ * 5], F32, tag=f"scrF_{lo}", name=f"scrF_{lo}"
                )
                nc.scalar.activation(
                    scr_f[:], fdiff[:], AF.Abs,
                    accum_out=fs[:, acc_col : acc_col + 1],
                )

            o4s: dict = {}
            s4s: dict = {}
            d4s: dict = {}
            oa4s: dict = {}
            ats: dict = {}
            dts: dict = {}

            for i in range(nit + 5):
                # ---- Pool (data i-2): d = out - scaled as f16 ----
                j = i - 2
                if 0 <= j < nit:
                    hw = TPI * N_DOS // 2
                    d4 = wk_pool.tile([128, TPI * N_DOS], F16, tag="d4")
                    oa4 = wk_pool.tile([128, TPI * N_DOS], F16, tag="oa4")
                    for hh in range(2):
                        sl = slice(hh * hw, (hh + 1) * hw)
                        nc.gpsimd.tensor_tensor(
                            d4[:, sl], o4s[j][:, sl], s4s[j][:, sl],
                            op=ALU.subtract,
                        )
                        nc.scalar.activation(oa4[:, sl], o4s[j][:, sl], AF.Abs)
                    d4s[j] = d4
                    oa4s[j] = oa4

                # ---- PE transposes (data i-3) + ACT/DVE evacuations ----
                k = i - 3
                if 0 <= k < nit:
                    oa4 = oa4s[k]
                    a_ts = []
                    for t in range(TPI):
                        p_o = pso_pool.tile([128, 512], F16, tag="psO")
                        for cc in range(NCH):
                            kk = _chunk_rows(cc)
                            nc.tensor.transpose(
                                p_o[0:kk, cc * 128 : cc * 128 + 128],
                                oa4[
                                    :,
                                    t * N_DOS + 128 * cc : t * N_DOS + 128 * cc + kk,
                                ],
                                i16[:],
                            )
                        a_t = at_pool.tile([128, 512], F16, tag="a_t")
                        if t < 3:
                            nc.scalar.copy(a_t[:], p_o[:])
                        else:
                            nc.vector.tensor_copy(a_t[:], p_o[:])
                        a_ts.append(a_t)
                    ats[k] = a_ts
                    d4 = d4s[k]
                    d_ts = []
                    for t in range(TPI):
                        p_d = psd_pool.tile([128, 512], F16, tag="psD")
                        for cc in range(NCH):
                            kk = _chunk_rows(cc)
                            nc.tensor.transpose(
                                p_d[0:kk, cc * 128 : cc * 128 + 128],
                                d4[:, t * N_DOS + 128 * cc : t * N_DOS + 128 * cc + kk],
                                i16[:],
                            )
                        d_t = dt_pool.tile([128, 512], F16, tag="d_t")
                        nc.vector.tensor_copy(d_t[:], p_d[:])
                        d_ts.append(d_t)
                    dts[k] = d_ts

                # ---- PE matmuls + reduces (data i-4) ----
                m = i - 4
                if 0 <= m < nit:
                    a_ts = ats.pop(m)
                    ps6 = ps6_pool.tile([128, TPI * 6], F32, tag="ps6")
                    for t in range(TPI):
                        for cc in range(NCH):
                            kk = _chunk_rows(cc)
                            nc.tensor.matmul(
                                ps6[:, 6 * t : 6 * t + 6],
                                lhsT=a_ts[t][0:kk, cc * 128 : cc * 128 + 128],
                                rhs=w128[0:kk, 6 * cc : 6 * cc + 6],
                                start=(cc == 0),
                                stop=(cc == NCH - 1),
                            )
                    d_ts = dts.pop(m)
                    cpss = []
                    for t in range(TPI):
                        c_ps = cps_pool.tile([128, N_DOS], F32, tag="cps")
                        for cc in range(NCH):
                            kk = _chunk_rows(cc)
                            lo = 128 * cc
                            nc.tensor.matmul(
                                c_ps[:, lo:N_DOS],
                                lhsT=d_ts[t][0:kk, cc * 128 : cc * 128 + 128],
                                rhs=u128[0:kk, cc * N_DOS + lo : (cc + 1) * N_DOS],
                                start=(cc == 0),
                                stop=(cc == NCH - 1),
                                skip_group_check=True,
                            )
                        cpss.append(c_ps)
                    nc.scalar.copy(s6[:, 6 * TPI * m : 6 * TPI * (m + 1)], ps6[:])
                    for t in range(TPI):
                        col = TPI * m + t
                        if t < 2:
                            scr_a = scr_pool.tile(
                                [128, N_DOS], F16, tag="scrA", name="scrA"
                            )
                            nc.scalar.activation(
                                scr_a[:],
                                cpss[t][:],
                                AF.Abs,
                                accum_out=csums[:, col : col + 1],
                            )
                        else:
                            nc.vector.tensor_reduce(
                                csums[:, col : col + 1],
                                cpss[t][:],
                                axis=AX.X,
                                op=ALU.add,
                                apply_absolute_value=True,
                            )
                    d4v = d4s[m][:].rearrange("q (j n) -> q j n", j=TPI)
                    nc.vector.tensor_reduce(
                        dsums[:, m : m + 1],
                        d4v[:],
                        axis=AX.XY,
                        op=ALU.add,
                        apply_absolute_value=True,
                    )
                    d4s.pop(m, None)
                    o4s.pop(m, None)
                    s4s.pop(m, None)
                    oa4s.pop(m, None)

                if i == nit - 3:
                    emit_aux_loads()

                # ---- loads (half-tensor granularity for latency) ----
                if i < nit:
                    o4 = io_pool.tile([128, TPI * N_DOS], F32, tag="o4")
                    s4 = io_pool.tile([128, TPI * N_DOS], F32, tag="s4")
                    hw2 = TPI * N_DOS // 2
                    for hh in range(2):
                        sl = slice(hh * hw2, (hh + 1) * hw2)
                        nc.sync.dma_start(o4[:, sl], src_o[:, i, sl])
                        nc.sync.dma_start(s4[:, sl], src_s[:, i, sl])
                    o4s[i] = o4
                    s4s[i] = s4

            # ---- end phase: feature math ----
            emit_feature_math(0, nt, 2)

            dsc = fin_pool.tile([128, nt], F32, tag="dsc")
            nc.vector.tensor_tensor(dsc[:], sc_c[:], sf_c[:], op=ALU.subtract)
            scr_s = fin_pool.tile([128, nt], F32, tag="scrS")
            nc.scalar.activation(
                scr_s[:], dsc[:], AF.Abs, accum_out=fs[:, 3:4]
            )

            nc.vector.tensor_reduce(fs[:, 0:1], dsums[:], axis=AX.X, op=ALU.add)
            nc.vector.tensor_reduce(fs[:, 1:2], csums[:], axis=AX.X, op=ALU.add)
            nc.gpsimd.memset(fs[:, 4:8], 0.0)

            nc.sync.dma_start(d_res[:], fs[:])

    nc.compile()
    return nc


_NC_CACHE: dict = {}


def _get_nc(rows: int) -> bass.Bass:
    if rows not in _NC_CACHE:
        _NC_CACHE[rows] = build_nc(rows)
    return _NC_CACHE[rows]


def make_in_maps(out, scaling, scaled, scaling_factor, features, n_cores=N_CORES):
    rows = out.shape[0] // n_cores
    w128 = _w128_np()
    u128 = _u128_np()
    ident = np.eye(128, dtype=np.float32)
    i16 = np.eye(128, dtype=np.float16)
    in_maps = []
    for i in range(n_cores):
        sl = slice(i * rows, (i + 1) * rows)
        in_maps.append(
            {
                "x_out": np.ascontiguousarray(out[sl]),
                "x_scaled": np.ascontiguousarray(scaled[sl]),
                "x_scaling": np.ascontiguousarray(scaling[sl]),
                "x_sf": np.ascontiguousarray(scaling_factor[sl]),
                "x_feat": np.ascontiguousarray(features[sl]),
                "w128_const": w128,
                "u128_const": u128,
                "ident": ident,
                "ident16": i16,
            }
        )
    return in_maps


def combine_partials(partials_list, b_full: int) -> np.float32:
    tot = np.zeros(5, np.float64)
    for fs in partials_list:
        tot += fs[:, 0:5].astype(np.float64).sum(axis=0)
    dos_loss = tot[0] / (b_full * N_DOS)
    cumsum_loss = tot[1] / (b_full * N_DOS)
    features_loss = (tot[2] + tot[4]) / (b_full * 5)
    scaling_loss = tot[3] / b_full
    return np.float32(
        dos_loss
        + SCALING_W * scaling_loss
        + CUMSUM_W * cumsum_loss
        + FEATURES_W * features_loss
    )


def kernel(out, scaling, scaled, scaling_factor, features):
    out = np.asarray(out, np.float32)
    scaling = np.asarray(scaling, np.float32)
    scaled = np.asarray(scaled, np.float32)
    scaling_factor = np.asarray(scaling_factor, np.float32)
    features = np.asarray(features, np.float32)

    nc = _get_nc(ROWS)
    in_maps = make_in_maps(out, scaling, scaled, scaling_factor, features)
    res = run_bass_kernel_spmd(nc, in_maps, list(range(N_CORES)))
    partials = [res.results[i]["partials"] for i in range(N_CORES)]
    return combine_partials(partials, out.shape[0])


if __name__ == "__main__":
    print("building...")
    nc = build_nc(2048)
    print("instructions built ok")
